# revision 14
# baseline (speedup 1.0000x reference)
"""Distributed multi-head attention (QKV proj + RoPE + softmax attention + out proj)
on 8 TRN2 NeuronCores.

Sharding: tensor-parallel over heads. Core c owns heads (2c, 2c+1):
  - qkv^T = W_c @ x^T for its 384 channels over all 4096 tokens (bf16 matmul)
  - RoPE on q,k (fp32, partition-swap via SBUF-SBUF DMA)
  - scores^T = k @ q^T per (batch, head): both heads' scores go into one
    2-bank PSUM tile (row-tiled matmuls), one exp [128,1024] on ScalarE
  - ctx^T = [v | 1] @ expS^T : M=65 matmul computes context + softmax denominator
  - per-qt normalization: ctx copied out of PSUM on DVE+GpSimd in parallel,
    reciprocal on DVE, partition-broadcast on GpSimd, normalize mults on DVE
  - per-batch AllToAll redistributes ctx: head-sharded -> token-sharded
    (256 tokens of each batch per core); b0's collective overlaps b1's
    attention, b1's collective overlaps b0's out-projection
  - out^T = W_out^T.T @ ctx_full^T + b_out per 256-token half

Host side: transposes/shards weights, runs SPMD, gathers [1024, 512] fp32 per core
(256 b0 tokens | 256 b1 tokens), reassembles to [2, 2048, 1024].
"""

import numpy as np
import ml_dtypes

import concourse.bass as bass
import concourse.tile as tile
from concourse import bacc, mybir
from concourse.bass_utils import run_bass_kernel_spmd
from concourse.masks import make_identity

BF16 = ml_dtypes.bfloat16

B, L, D, H, Hd = 2, 2048, 1024, 16, 64
T = B * L              # 4096 tokens
NC = 8                 # cores
HPC = H // NC          # 2 heads per core
TOK = T // NC          # 512 token shard per core (256 from each batch)
HTOK = TOK // 2        # 256
NT = T // 512          # 8 token n-tiles of 512
KT = L // 128          # 16 k-tiles per batch
QT = L // 512          # 4 q-tiles per batch

F32 = mybir.dt.float32
BF = mybir.dt.bfloat16


def build(debug=False):
    nc = bacc.Bacc(None, target_bir_lowering=False, num_devices=NC)
    dbg = {}
    if debug:
        for nm, shp in [("dbg_q", [128, 512]), ("dbg_k", [128, 512]),
                        ("dbg_v", [128, 512]), ("dbg_cu0", [65, 512]),
                        ("dbg_cu1", [65, 512]), ("dbg_rc", [2, 512]),
                        ("dbg_bca0", [64, 512]), ("dbg_cn", [128, 512]),
                        ("dbg_cf", [128, 256])]:
            dbg[nm] = nc.dram_tensor(nm, shp, F32, kind="ExternalOutput")

    xT = nc.dram_tensor("xT", [D, T], BF, kind="ExternalInput")          # x^T, replicated
    wq = nc.dram_tensor("wqkvT", [D, 3 * 128], BF, kind="ExternalInput")  # W_c^T per core
    bq = nc.dram_tensor("bqkv", [128, 3], F32, kind="ExternalInput")      # bias cols q,k,v
    cosT = nc.dram_tensor("cosT", [128, L], F32, kind="ExternalInput")
    sinT = nc.dram_tensor("sinT", [128, L], F32, kind="ExternalInput")    # sign-folded sin
    wo = nc.dram_tensor("woutT", [D, D], BF, kind="ExternalInput")        # W_out^T, replicated
    bo = nc.dram_tensor("bout", [128, NC], F32, kind="ExternalInput")
    out = nc.dram_tensor("out", [D, TOK], F32, kind="ExternalOutput")

    with tile.TileContext(nc) as tc:
        with tc.tile_pool(name="const", bufs=1) as const, \
             tc.tile_pool(name="big", bufs=1) as big, \
             tc.tile_pool(name="rope", bufs=3) as rope, \
             tc.tile_pool(name="es", bufs=6) as esp, \
             tc.tile_pool(name="cu", bufs=2) as cup, \
             tc.tile_pool(name="small", bufs=3) as small, \
             tc.tile_pool(name="psum", bufs=1, space="PSUM") as psum, \
             tc.tile_pool(name="dram", bufs=1, space="DRAM") as dram:

            # ---------------- constants / weights ----------------------------
            ident = const.tile([128, 128], BF, tag="ident")
            make_identity(nc, ident[:])

            # one consolidated weight DMA on the sync queue, before x
            w_sb = const.tile([128, 8, 3 * 128], BF, tag="w")
            wq_ap = wq[:]
            w_src = bass.AP(
                tensor=wq_ap.tensor, offset=wq_ap.offset,
                ap=[[3 * 128, 128], [128 * 3 * 128, 8], [1, 3 * 128]])
            nc.sync.dma_start(w_sb[:], w_src)

            # small/rope tables on the scalar queue (idle during prologue)
            bq_sb = const.tile([128, 3], F32, tag="bq")
            nc.scalar.dma_start(bq_sb[:], bq[:])
            cos_sb = const.tile([128, L], F32, tag="cos")
            nc.scalar.dma_start(cos_sb[:], cosT[:])
            sin_sb = const.tile([128, L], F32, tag="sin")
            nc.scalar.dma_start(sin_sb[:], sinT[:])
            bo_sb = const.tile([128, NC], F32, tag="bo")
            nc.scalar.dma_start(bo_sb[:], bo[:])

            qT_sb = big.tile([128, T], BF, tag="qT")
            kT_sb = big.tile([128, T], BF, tag="kT")
            v_sb = big.tile([128, T], BF, tag="v")
            # transposed v with a built-in ones column: [tok%128, blk, head, 65]
            vn_sb = big.tile([128, T // 128, HPC, 65], BF, tag="vn")
            nc.vector.memset(vn_sb[:, :, :, 64:65], 1.0)

            # x staging: one tile + one DMA per 512-token n-tile
            x_sb = [big.tile([128, 8, 512], BF, tag="xst", bufs=5,
                             name=f"x_{n}") for n in range(NT)]

            def load_x(n, eng):
                xa = xT[:]
                src = bass.AP(
                    tensor=xa.tensor, offset=xa.offset + 512 * n,
                    ap=[[T, 128], [128 * T, 8], [1, 512]])
                eng.dma_start(x_sb[n][:], src)

            for n in range(QT):
                load_x(n, nc.sync)

            wo_sb = const.tile([128, 8, D], BF, tag="wo")

            # per-batch all-to-all buffers (chunk j = 256 tokens for core j)
            a2a_in = [dram.tile([NC, 128, HTOK], BF, tag=f"a2a_in{b}",
                                name=f"a2a_in{b}") for b in range(B)]
            a2a_out = [dram.tile([NC, 128, HTOK], BF, tag=f"a2a_out{b}",
                                 name=f"a2a_out{b}") for b in range(B)]

            # ---------------- per-stage emitters ------------------------------
            def stage1_qkv_m(n, m):
                """QKV matmul + bias (+rope for q/k) for one (n-tile, m)."""
                ts = slice(512 * n, 512 * (n + 1))
                cs = slice(512 * (n % QT), 512 * (n % QT) + 512)
                ps = psum.tile([128, 512], F32, tag="ps1", bufs=2,
                               name=f"s1_{n}_{m}")
                for k in range(8):
                    nc.tensor.matmul(
                        ps[:],
                        w_sb[:, k, 128 * m:128 * (m + 1)],
                        x_sb[n][:, k, :],
                        start=(k == 0), stop=(k == 7),
                    )
                if m < 2:  # q or k: ACT evicts (+bias) fast to free the
                    # PSUM slot; rope split across DVE and GpSimd
                    dst = qT_sb if m == 0 else kT_sb
                    qb = rope.tile([128, 512], F32, tag="qb", bufs=4,
                                   name=f"qb_{n}_{m}")
                    nc.scalar.activation(
                        qb[:], ps[:],
                        mybir.ActivationFunctionType.Identity,
                        bias=bq_sb[:, m:m + 1])
                    qc = rope.tile([128, 512], F32, tag="qc", name=f"qc_{n}_{m}")
                    nc.vector.tensor_tensor(
                        qc[:], qb[:], cos_sb[:, cs], mybir.AluOpType.mult)
                    qs = rope.tile([128, 512], F32, tag="qs", name=f"qs_{n}_{m}")
                    nc.gpsimd.tensor_tensor(
                        qs[:], qb[:], sin_sb[:, cs], mybir.AluOpType.mult)
                    qw = rope.tile([128, 512], F32, tag="qw", name=f"qw_{n}_{m}")
                    for blk in range(4):
                        src = 32 * (blk ^ 1)
                        nc.gpsimd.dma_start(
                            qw[32 * blk:32 * blk + 32, :],
                            qs[src:src + 32, :])
                    nc.vector.tensor_tensor(
                        dst[:, ts], qc[:], qw[:], mybir.AluOpType.add)
                else:  # v: bias only, straight to bf16
                    nc.scalar.activation(
                        v_sb[:, ts], ps[:],
                        mybir.ActivationFunctionType.Identity,
                        bias=bq_sb[:, 2:3])

            def stage1_qkv(n):
                for m in range(3):
                    stage1_qkv_m(n, m)

            def stage1_vtr(j):
                """Transpose one 128-token block of v into vn (both heads)."""
                tp = psum.tile([128, 128], BF, tag="ps1", bufs=2, name=f"tr_{j}")
                nc.tensor.transpose(tp[:], v_sb[:, 128 * j:128 * (j + 1)], ident[:])
                for h in range(HPC):
                    nc.vector.tensor_copy(
                        vn_sb[:, j, h, 0:64], tp[:, 64 * h:64 * (h + 1)])

            def stage2_open(b, qt):
                return [psum.tile([65, 512], F32, tag=f"ctx{h}", bufs=1,
                                  name=f"ctx_{b}_{qt}_{h}")
                        for h in range(HPC)]

            def stage2_kts(b, qt, ctxs, kts, fill_iter):
                qsl = slice(2048 * b + 512 * qt, 2048 * b + 512 * qt + 512)
                for kt in kts:
                    ksl = slice(2048 * b + 128 * kt, 2048 * b + 128 * kt + 128)
                    blk = 16 * b + kt
                    st2 = psum.tile([128, 1024], F32, tag="st", bufs=2,
                                    name=f"st_{b}_{qt}_{kt}")
                    for h in range(HPC):
                        nc.tensor.matmul(
                            st2[:, 512 * h:512 * (h + 1)],
                            kT_sb[64 * h:64 * (h + 1), ksl],
                            qT_sb[64 * h:64 * (h + 1), qsl],
                            start=True, stop=True)
                    es = esp.tile([128, 1024], BF, tag="es",
                                  name=f"es_{b}_{qt}_{kt}")
                    nc.scalar.activation(
                        es[:], st2[:], mybir.ActivationFunctionType.Exp)
                    for h in range(HPC):
                        nc.tensor.matmul(
                            ctxs[h][:],
                            vn_sb[:, blk, h, :],
                            es[:, 512 * h:512 * (h + 1)],
                            start=(kt == 0), stop=(kt == KT - 1))
                    fill_iter(b, qt, kt)

            def dbg_dump(name, ap):
                if dbg:
                    t = small.tile(list(ap.shape), F32, tag="d_all",
                                   name=f"d_{name}", bufs=2,
                                   padded_shape=[128, 512])
                    nc.vector.tensor_copy(t[:], ap)
                    nc.sync.dma_start(dbg[name][:], t[:])

            def ctx_evict(b, qt, ctxs):
                # copy both heads out of PSUM in parallel (DVE + GpSimd) so the
                # ctx banks free fast, then normalize from SBUF
                cu0 = cup.tile([65, 512], F32, tag="cu0", name=f"cu_{b}_{qt}_0")
                nc.vector.tensor_copy(cu0[:], ctxs[0][:])
                cu1 = cup.tile([65, 512], F32, tag="cu1", name=f"cu_{b}_{qt}_1")
                nc.vector.tensor_copy(cu1[:], ctxs[1][:])
                # custom-DVE recip needs a partition-0 input: copy dens down
                dn0 = small.tile([1, 512], F32, tag="dn0",
                                 name=f"dn_{b}_{qt}_0", bufs=2)
                nc.vector.tensor_copy(dn0[:], cu0[64:65, :])
                dn1 = small.tile([1, 512], F32, tag="dn1",
                                 name=f"dn_{b}_{qt}_1", bufs=2)
                nc.vector.tensor_copy(dn1[:], cu1[64:65, :])
                rc0 = small.tile([1, 512], F32, tag="rc0",
                                 name=f"rc_{b}_{qt}_0", bufs=2)
                nc.vector.reciprocal_approx_fast(rc0[:], dn0[:])
                rc1 = small.tile([1, 512], F32, tag="rc1",
                                 name=f"rc_{b}_{qt}_1", bufs=2)
                nc.vector.reciprocal_approx_fast(rc1[:], dn1[:])
                bca0 = small.tile([64, 512], F32, tag="bca0",
                                  name=f"bca_{b}_{qt}_0", bufs=3)
                nc.gpsimd.partition_broadcast(bca0[:], rc0[:])
                bca1 = small.tile([64, 512], F32, tag="bca1",
                                  name=f"bca_{b}_{qt}_1", bufs=3)
                nc.gpsimd.partition_broadcast(bca1[:], rc1[:])
                if b == 0 and qt == 0:
                    dbg_dump("dbg_cu0", cu0[:])
                    dbg_dump("dbg_cu1", cu1[:])
                    dbg_dump("dbg_bca0", bca0[:])
                cn = small.tile([128, 512], BF, tag="cn", name=f"cn_{b}_{qt}")
                nc.vector.tensor_tensor(
                    cn[0:64, :], cu0[0:64, :], bca0[:],
                    mybir.AluOpType.mult)
                nc.vector.tensor_tensor(
                    cn[64:128, :], cu1[0:64, :], bca1[:],
                    mybir.AluOpType.mult)
                if b == 0 and qt == 0:
                    dbg_dump("dbg_cn", cn[:])
                # tokens 512*qt..512*qt+512 of batch b -> dest cores 2qt, 2qt+1
                for c in range(2):
                    nc.sync.dma_start(
                        a2a_in[b][2 * qt + c, :, :],
                        cn[:, 256 * c:256 * (c + 1)])

            def run_batch(b, fill_iter):
                for qt in range(QT):
                    ctxs = stage2_open(b, qt)
                    stage2_kts(b, qt, ctxs, range(KT), fill_iter)
                    ctx_evict(b, qt, ctxs)

            def emit_a2a(b):
                nc.gpsimd.collective_compute(
                    "AllToAll",
                    mybir.AluOpType.bypass,
                    replica_groups=[list(range(NC))],
                    ins=[a2a_in[b].opt()],
                    outs=[a2a_out[b].opt()],
                )

            ctxf_sb = [big.tile([128, HTOK], BF, tag=f"cf{b}_{k}",
                                name=f"cf{b}_{k}")
                       for b in range(B) for k in range(8)]

            def load_ctx(b):
                for k in range(8):
                    (nc.sync if k % 2 == 0 else nc.gpsimd).dma_start(
                        ctxf_sb[8 * b + k][:], a2a_out[b][k, :, :])

            def stage4(b):
                for m in range(8):
                    pso = psum.tile([128, HTOK], F32, tag="ps1", bufs=2,
                                    name=f"o_{b}_{m}")
                    for k in range(8):
                        nc.tensor.matmul(
                            pso[:],
                            wo_sb[:, k, 128 * m:128 * (m + 1)],
                            ctxf_sb[8 * b + k][:],
                            start=(k == 0), stop=(k == 7))
                    os_t = small.tile([128, HTOK], F32, tag="os",
                                      name=f"os_{b}_{m}", bufs=4)
                    nc.scalar.activation(
                        os_t[:], pso[:],
                        mybir.ActivationFunctionType.Identity,
                        bias=bo_sb[:, m:m + 1])
                    (nc.sync if m % 2 == 0 else nc.gpsimd).dma_start(
                        out[128 * m:128 * (m + 1), HTOK * b:HTOK * (b + 1)],
                        os_t[:])

            # ---------------- emission schedule -------------------------------
            # stage 1 for batch 0 (transposes follow each n-tile's v)
            for n in range(QT):
                stage1_qkv(n)
                for j in range(4 * n, 4 * n + 4):
                    stage1_vtr(j)

            # stage 2 for batch 0, with stage-1(b=1) units drip-fed to keep PE busy
            b1_units = []
            for n in range(QT, NT):
                b1_units.append(lambda n=n: load_x(n, nc.sync))
                for m in range(3):
                    b1_units.append(lambda n=n, m=m: stage1_qkv_m(n, m))
                for j in range(4 * n, 4 * n + 4):
                    b1_units.append(lambda j=j: stage1_vtr(j))
            # weights for the out projection arrive during batch-0 attention
            wo_ap = wo[:]
            wo_src = bass.AP(
                tensor=wo_ap.tensor, offset=wo_ap.offset,
                ap=[[D, 128], [128 * D, 8], [1, D]])
            b1_units.insert(5, lambda: nc.sync.dma_start(wo_sb[:], wo_src))
            unit_idx = [0]
            count = [0]
            # 64 kt-iterations in batch 0; ~33 fill units -> every 2nd iteration
            def fill_iter(b, qt, kt):
                count[0] += 1
                if b == 0 and count[0] % 2 == 0 and unit_idx[0] < len(b1_units):
                    b1_units[unit_idx[0]]()
                    unit_idx[0] += 1

            if dbg:
                dbg_dump("dbg_q", qT_sb[:, 0:512])
                dbg_dump("dbg_k", kT_sb[:, 0:512])
                dbg_dump("dbg_v", v_sb[:, 0:512])
            run_batch(0, fill_iter)
            while unit_idx[0] < len(b1_units):
                b1_units[unit_idx[0]]()
                unit_idx[0] += 1

            emit_a2a(0)  # overlaps batch-1 attention

            run_batch(1, lambda b, qt, kt: None)

            # batch-0 ctx is long since available; load + project while the
            # batch-1 collective runs
            load_ctx(0)
            emit_a2a(1)
            stage4(0)
            load_ctx(1)
            stage4(1)
            if dbg:
                dbg_dump("dbg_cf", ctxf_sb[0][:])

    nc.compile()
    return nc


_NC_CACHE = None


def _get_nc():
    global _NC_CACHE
    if _NC_CACHE is None:
        _NC_CACHE = build()
    return _NC_CACHE


def _host_prep(x, W_qkv, b_qkv, W_out, b_out):
    x = np.asarray(x, dtype=np.float32)
    W_qkv = np.asarray(W_qkv, dtype=np.float32)
    b_qkv = np.asarray(b_qkv, dtype=np.float32)
    W_out = np.asarray(W_out, dtype=np.float32)
    b_out = np.asarray(b_out, dtype=np.float32)

    scale = 1.0 / np.sqrt(Hd)
    xT = np.ascontiguousarray(x.reshape(T, D).T).astype(BF16)

    # rope tables (token position within batch), channel-transposed + sign-folded
    inv_freq = 1.0 / (10000.0 ** (np.arange(0, Hd, 2, dtype=np.float32) / Hd))  # [32]
    t_pos = np.arange(L, dtype=np.float32)
    freqs = np.outer(t_pos, inv_freq)                       # [L, 32]
    emb = np.concatenate([freqs, freqs], axis=1)            # [L, 64]
    cos_t = np.cos(emb).T.astype(np.float32)                # [64, L]
    sin_t = np.sin(emb).T.astype(np.float32)                # [64, L]
    sin2 = sin_t.copy()
    sin2[32:, :] *= -1.0                                    # s''[d] = +sin d<32, -sin d>=32
    cosT = np.ascontiguousarray(np.tile(cos_t, (2, 1)))     # [128, L]
    sinT = np.ascontiguousarray(np.tile(sin2, (2, 1)))

    woutT = np.ascontiguousarray(W_out.T).astype(BF16)      # [D, D]
    bo_sb = np.ascontiguousarray(b_out.reshape(NC, 128).T)  # [128, 8]

    in_maps = []
    for c in range(NC):
        r = slice(128 * c, 128 * (c + 1))
        Wq = W_qkv[0 * D:1 * D][r] * scale
        Wk = W_qkv[1 * D:2 * D][r]
        Wv = W_qkv[2 * D:3 * D][r]
        Wc = np.concatenate([Wq, Wk, Wv], axis=0)           # [384, 1024]
        WcT = np.ascontiguousarray(Wc.T).astype(BF16)       # [1024, 384]
        bq_c = np.stack([
            b_qkv[0 * D:1 * D][r] * scale,
            b_qkv[1 * D:2 * D][r],
            b_qkv[2 * D:3 * D][r],
        ], axis=1).astype(np.float32)                       # [128, 3]
        in_maps.append({
            "xT": xT,
            "wqkvT": WcT,
            "bqkv": np.ascontiguousarray(bq_c),
            "cosT": cosT,
            "sinT": sinT,
            "woutT": woutT,
            "bout": bo_sb,
        })
    return in_maps


def kernel_run(inputs, trace=False, tmpdir=None):
    nc = _get_nc()
    in_maps = _host_prep(**inputs)
    res = run_bass_kernel_spmd(
        nc, in_maps, list(range(NC)), trace=trace, tmpdir=tmpdir)
    outT = np.empty((D, T), dtype=np.float32)
    for c in range(NC):
        r = np.asarray(res.results[c]["out"], dtype=np.float32)  # [1024, 512]
        outT[:, HTOK * c:HTOK * (c + 1)] = r[:, 0:HTOK]
        outT[:, 2048 + HTOK * c:2048 + HTOK * (c + 1)] = r[:, HTOK:]
    out = np.ascontiguousarray(outT.T).reshape(B, L, D)
    return out, res


def kernel(**inputs):
    out, _ = kernel_run(inputs, trace=False)
    return out
